# revision 1
# baseline (speedup 1.0000x reference)
"""TRN2 Bass kernel for nn_CoreAttention_34875134444341.

Strategy (8 NeuronCores, no collectives):
  - Data-parallel over batch (4) x causal-balanced query-row split (2).
  - Each core: Q projection for its 1024 query rows (zig-zag tile
    assignment balances causal attention work) spilled to DRAM scratch;
    full K/V projections for its batch kept resident in SBUF; block
    attention in "transposed" orientation (keys on the partition axis)
    so no on-chip transposes are needed; final Wo matmul row-parallel
    (no reduction across cores).
  - All matmuls run as float32r (full-rate fp32 on the PE array) except
    the Wo stage which runs bf16.
  - Host gathers per-core outputs and inverse-permutes rows.
"""

import sys

sys.path.insert(0, "/opt/trn_rl_repo")

import numpy as np
import ml_dtypes

B, S, D = 4, 2048, 2048
H, HKV, DK = 16, 4, 128
RQ = RKV = 512
GROUP = H // HKV
P = 128

TILE_R = 256  # query rows per slot
KB = 128  # keys per block
NB_SCHED = [16, 12, 8, 4]  # key blocks per slot (same on every core)
TILE_ASSIGN = {0: [7, 5, 2, 0], 1: [6, 4, 3, 1]}  # slot -> query tile

ROWS_PER_CORE = 4 * TILE_R  # 1024

_CACHE = {}
TRACE = False
LAST_RESULT = None


def _rows_sched(parity):
    return np.concatenate(
        [np.arange(t * TILE_R, (t + 1) * TILE_R) for t in TILE_ASSIGN[parity]]
    )


def _make_mask(parity):
    """[128 key_in_block, 4 slots, 4 blocks, 512 (same mask for 2 heads)]."""
    m = np.zeros((KB, 4, 4, TILE_R), np.float32)
    for s in range(4):
        t = TILE_ASSIGN[parity][s]
        nb = NB_SCHED[s]
        row_g = t * TILE_R + np.arange(TILE_R)
        for j in range(4):  # last four blocks of the slot's schedule
            blk = nb - 4 + j
            key_g = blk * KB + np.arange(KB)
            bad = key_g[:, None] > row_g[None, :]
            m[:, s, j][bad] = -1e30
    return np.concatenate([m, m], axis=-1)


def _build_nc():
    import concourse.tile as tile
    from concourse import bacc, mybir

    f32 = mybir.dt.float32
    f32r = mybir.dt.float32r
    bf16 = mybir.dt.bfloat16
    Exp = mybir.ActivationFunctionType.Exp
    Mult = mybir.AluOpType.mult
    Add = mybir.AluOpType.add

    nc = bacc.Bacc("TRN2", target_bir_lowering=False, debug=False)

    xTkv = nc.dram_tensor("xTkv", [D, S], f32r, kind="ExternalInput")
    xTq = nc.dram_tensor("xTq", [D, ROWS_PER_CORE], f32r, kind="ExternalInput")
    wq1 = nc.dram_tensor("wq1", [D, RQ], f32r, kind="ExternalInput")
    wq2 = nc.dram_tensor("wq2", [RQ, H * DK], f32r, kind="ExternalInput")
    wk1 = nc.dram_tensor("wk1", [D, RKV], f32r, kind="ExternalInput")
    wk2 = nc.dram_tensor("wk2", [RKV, HKV * DK], f32r, kind="ExternalInput")
    wv1 = nc.dram_tensor("wv1", [D, RKV], f32r, kind="ExternalInput")
    wv2 = nc.dram_tensor("wv2", [RKV, HKV * DK], f32r, kind="ExternalInput")
    wo = nc.dram_tensor("wo", [D, D], bf16, kind="ExternalInput")
    maskin = nc.dram_tensor("maskin", [KB, 4, 4, 2 * TILE_R], f32, kind="ExternalInput")
    ones_in = nc.dram_tensor("ones_in", [P, 1], f32r, kind="ExternalInput")
    out = nc.dram_tensor("out", [ROWS_PER_CORE, D], f32, kind="ExternalOutput")

    qT_dram = nc.dram_tensor("qT_scratch", [H, P, ROWS_PER_CORE], f32r)

    xTkv_t = xTkv.rearrange("(dc p) s -> p dc s", p=P)  # [128, 16, 2048]
    xTq_t = xTq.rearrange("(dc p) r -> p dc r", p=P)  # [128, 16, 1024]
    wq1_t = wq1.rearrange("(dc p) r -> p dc r", p=P)  # [128, 16, 512]
    wk1_t = wk1.rearrange("(dc p) r -> p dc r", p=P)
    wv1_t = wv1.rearrange("(dc p) r -> p dc r", p=P)
    wq2_t = wq2.rearrange("(rc p) h -> p rc h", p=P)  # [128, 4, 2048]
    wk2_t = wk2.rearrange("(rc p) h -> p rc h", p=P)  # [128, 4, 512]
    wv2_t = wv2.rearrange("(rc p) h -> p rc h", p=P)
    wo_t = wo.rearrange("(hc p) o -> p hc o", p=P)  # [128, 16, 2048]

    with tile.TileContext(nc) as tc:
        with tc.tile_pool(name="persist", bufs=1) as persist:
            ones_sb = persist.tile([P, 1], f32r)
            nc.sync.dma_start(ones_sb[:], ones_in[:])

            # ------- Phase 1: Q projection -> DRAM scratch ----------------
            with (
                tc.tile_pool(name="q_w", bufs=1) as q_w,
                tc.tile_pool(name="q_x", bufs=2) as q_x,
                tc.tile_pool(name="q_mid", bufs=1) as q_mid,
                tc.tile_pool(name="q_out", bufs=3) as q_out,
                tc.tile_pool(name="q_ps", bufs=4, space="PSUM") as q_ps,
            ):
                wq1_sb = q_w.tile([P, 16, RQ], f32r)
                nc.sync.dma_start(wq1_sb[:], wq1_t)
                xts = []
                for tcn in range(2):
                    xt = q_x.tile([P, 16, 512], f32r, tag="xtq")
                    nc.sync.dma_start(xt[:], xTq_t[:, :, tcn * 512 : (tcn + 1) * 512])
                    xts.append(xt)
                wq2_sb = q_w.tile([P, 4, H * DK], f32r)
                nc.sync.dma_start(wq2_sb[:], wq2_t)

                q1t = q_mid.tile([P, 4, ROWS_PER_CORE], f32r)
                for tcn in range(2):
                    xt = xts[tcn]
                    for rc in range(4):
                        ps_q = q_ps.tile([P, 512], f32, tag="psq1")
                        for dc in range(16):
                            nc.tensor.matmul(
                                ps_q[:],
                                wq1_sb[:, dc, rc * P : (rc + 1) * P],
                                xt[:, dc],
                                start=(dc == 0),
                                stop=(dc == 15),
                            )
                        nc.any.tensor_copy(
                            q1t[:, rc, tcn * 512 : (tcn + 1) * 512], ps_q[:]
                        )
                for h in range(H):
                    for tcn in range(2):
                        ps_qT = q_ps.tile([P, 512], f32, tag="psq2")
                        for rc in range(4):
                            nc.tensor.matmul(
                                ps_qT[:],
                                wq2_sb[:, rc, h * P : (h + 1) * P],
                                q1t[:, rc, tcn * 512 : (tcn + 1) * 512],
                                start=(rc == 0),
                                stop=(rc == 3),
                            )
                        qbounce = q_out.tile([P, 512], f32r, tag="qb")
                        nc.any.tensor_copy(qbounce[:], ps_qT[:])
                        nc.sync.dma_start(
                            qT_dram[h, :, tcn * 512 : (tcn + 1) * 512], qbounce[:]
                        )

            # ------- kT/v stay resident from here on ----------------------
            with tc.tile_pool(name="kv_keep", bufs=1) as kv_keep:
                kT_sb = kv_keep.tile([P, HKV, S], f32r)
                v_sb = kv_keep.tile([P, S // P, HKV * DK], f32r)

                # ---- Phase 2: K then V projections (resident outputs) ----
                with (
                    tc.tile_pool(name="kv_w1", bufs=1) as kv_w1,
                    tc.tile_pool(name="kv_w2", bufs=1) as kv_w2,
                    tc.tile_pool(name="kv_x", bufs=2) as kv_x,
                    tc.tile_pool(name="kv_mid", bufs=1) as kv_mid,
                    tc.tile_pool(name="kv_ps", bufs=4, space="PSUM") as kv_ps,
                ):
                    for which in range(2):  # 0 = K, 1 = V
                        w1_t, w2_t = (wk1_t, wk2_t) if which == 0 else (wv1_t, wv2_t)
                        w1_sb = kv_w1.tile([P, 16, RKV], f32r, tag="w1")
                        for dq in range(4):
                            nc.sync.dma_start(
                                w1_sb[:, dq * 4 : (dq + 1) * 4], w1_t[:, dq * 4 : (dq + 1) * 4]
                            )
                        w2_sb = kv_w2.tile([P, 4, HKV * DK], f32r, tag="w2")
                        nc.sync.dma_start(w2_sb[:], w2_t)

                        for tcn in range(4):  # token 512-chunks
                            xt = kv_x.tile([P, 16, 512], f32r, tag="xt")
                            nc.sync.dma_start(
                                xt[:], xTkv_t[:, :, tcn * 512 : (tcn + 1) * 512]
                            )
                            mid = kv_mid.tile([P, 4, 512], f32r, tag="mid")
                            for rc in range(4):
                                ps_1 = kv_ps.tile([P, 512], f32, tag="ps1")
                                for dc in range(16):
                                    nc.tensor.matmul(
                                        ps_1[:],
                                        w1_sb[:, dc, rc * P : (rc + 1) * P],
                                        xt[:, dc],
                                        start=(dc == 0),
                                        stop=(dc == 15),
                                    )
                                nc.any.tensor_copy(mid[:, rc], ps_1[:])

                            if which == 0:
                                for hc in range(HKV):
                                    ps_2 = kv_ps.tile([P, 512], f32, tag="ps2")
                                    for rc in range(4):
                                        nc.tensor.matmul(
                                            ps_2[:],
                                            w2_sb[:, rc, hc * P : (hc + 1) * P],
                                            mid[:, rc],
                                            start=(rc == 0),
                                            stop=(rc == 3),
                                        )
                                    nc.any.tensor_copy(
                                        kT_sb[:, hc, tcn * 512 : (tcn + 1) * 512],
                                        ps_2[:],
                                    )
                            else:
                                for i in range(4):
                                    ps_2 = kv_ps.tile([P, 512], f32, tag="ps2")
                                    for rc in range(4):
                                        nc.tensor.matmul(
                                            ps_2[:],
                                            mid[:, rc, i * P : (i + 1) * P],
                                            w2_sb[:, rc],
                                            start=(rc == 0),
                                            stop=(rc == 3),
                                        )
                                    nc.any.tensor_copy(v_sb[:, tcn * 4 + i], ps_2[:])

                # ---- Phases 3+4 share attn_all ----
                with tc.tile_pool(name="attn_keep", bufs=1) as attn_keep:
                    attn_all = attn_keep.tile([P, 4, H, TILE_R], bf16)

                    with (
                        tc.tile_pool(name="at_m", bufs=1) as at_m,
                        tc.tile_pool(name="at_q", bufs=2) as at_q,
                        tc.tile_pool(name="at_e", bufs=4) as at_e,
                        tc.tile_pool(name="at_small", bufs=4) as at_small,
                        tc.tile_pool(name="at_ps", bufs=4, space="PSUM") as at_ps,
                        tc.tile_pool(name="at_ps_acc", bufs=2, space="PSUM") as at_ps_acc,
                        tc.tile_pool(name="at_ps_sum", bufs=2, space="PSUM") as at_ps_sum,
                    ):
                        mask_sb = at_m.tile([P, 4, 4, 2 * TILE_R], f32)
                        nc.sync.dma_start(mask_sb[:], maskin[:])

                        for s in (3, 2, 1, 0):
                            nb = NB_SCHED[s]
                            qT_sl = at_q.tile([P, H, TILE_R], f32r, tag="qsl")
                            nc.sync.dma_start(
                                qT_sl[:],
                                qT_dram.rearrange("h p r -> p h r")[
                                    :, :, s * TILE_R : (s + 1) * TILE_R
                                ],
                            )
                            for hp in range(H // 2):  # head pairs share kvh
                                h0 = 2 * hp
                                kvh = h0 // GROUP
                                # packed accumulator for both heads; only the
                                # first MM carries start=True (bank-wide clear)
                                ps_at = at_ps_acc.tile([P, 2 * TILE_R], f32, tag="at")
                                ps_sum = at_ps_sum.tile(
                                    [1, 2 * TILE_R], f32, tag="sum"
                                )
                                for b in range(nb):
                                    # one kT-block LDWEIGHTS feeds both heads
                                    ps_sc = at_ps.tile([P, 2 * TILE_R], f32, tag="sc")
                                    for i in range(2):
                                        nc.tensor.matmul(
                                            ps_sc[:, i * TILE_R : (i + 1) * TILE_R],
                                            kT_sb[:, kvh, b * KB : (b + 1) * KB],
                                            qT_sl[:, h0 + i],
                                            start=True,
                                            stop=True,
                                        )
                                    j = b - (nb - 4)
                                    if j >= 0:
                                        nc.vector.tensor_tensor(
                                            ps_sc[:], ps_sc[:], mask_sb[:, s, j], Add
                                        )
                                    e_sb = at_e.tile([P, 2 * TILE_R], f32r, tag="e")
                                    nc.scalar.activation(e_sb[:], ps_sc[:], Exp)
                                    for i in range(2):
                                        nc.tensor.matmul(
                                            ps_at[:, i * TILE_R : (i + 1) * TILE_R],
                                            v_sb[:, b, kvh * DK : (kvh + 1) * DK],
                                            e_sb[:, i * TILE_R : (i + 1) * TILE_R],
                                            start=(b == 0 and i == 0),
                                            stop=(b == nb - 1),
                                        )
                                    nc.tensor.matmul(
                                        ps_sum[:],
                                        ones_sb[:],
                                        e_sb[:],
                                        start=(b == 0),
                                        stop=(b == nb - 1),
                                    )
                                rec_sb = at_small.tile([1, 2 * TILE_R], f32, tag="rec")
                                nc.vector.reciprocal(rec_sb[:], ps_sum[:])
                                bc_sb = at_small.tile([P, 2 * TILE_R], f32, tag="bc")
                                nc.gpsimd.partition_broadcast(bc_sb[:], rec_sb[:])
                                for i in range(2):
                                    nc.vector.tensor_tensor(
                                        attn_all[:, s, h0 + i],
                                        ps_at[:, i * TILE_R : (i + 1) * TILE_R],
                                        bc_sb[:, i * TILE_R : (i + 1) * TILE_R],
                                        Mult,
                                    )

                    # ---- Phase 4: Wo ----
                    with (
                        tc.tile_pool(name="wo_w", bufs=2) as wo_w,
                        tc.tile_pool(name="wo_out", bufs=3) as wo_out,
                        tc.tile_pool(name="wo_ps", bufs=3, space="PSUM") as wo_ps,
                    ):
                        for oc in range(4):
                            wo_sb = wo_w.tile([P, 16, 512], bf16, tag="woc")
                            nc.sync.dma_start(
                                wo_sb[:], wo_t[:, :, oc * 512 : (oc + 1) * 512]
                            )
                            for rc in range(8):
                                s, half = rc // 2, rc % 2
                                ps_o = wo_ps.tile([P, 512], f32, tag="o")
                                for hc in range(16):
                                    nc.tensor.matmul(
                                        ps_o[:],
                                        attn_all[
                                            :, s, hc, half * P : (half + 1) * P
                                        ],
                                        wo_sb[:, hc],
                                        start=(hc == 0),
                                        stop=(hc == 15),
                                    )
                                o_sb = wo_out.tile([P, 512], f32, tag="osb")
                                nc.vector.tensor_copy(o_sb[:], ps_o[:])
                                nc.sync.dma_start(
                                    out[
                                        rc * P : (rc + 1) * P,
                                        oc * 512 : (oc + 1) * 512,
                                    ],
                                    o_sb[:],
                                )

    nc.finalize()
    return nc


def kernel(x, Wq1, Wq2, Wk1, Wk2, Wv1, Wv2, Wo):
    global LAST_RESULT
    from concourse.bass_utils import run_bass_kernel_spmd

    x = np.asarray(x, dtype=np.float32)
    Wq1 = np.asarray(Wq1, dtype=np.float32)
    Wq2 = np.asarray(Wq2, dtype=np.float32)
    Wk1 = np.asarray(Wk1, dtype=np.float32)
    Wk2 = np.asarray(Wk2, dtype=np.float32)
    Wv1 = np.asarray(Wv1, dtype=np.float32)
    Wv2 = np.asarray(Wv2, dtype=np.float32)
    Wo = np.asarray(Wo, dtype=np.float32)

    if "nc" not in _CACHE:
        _CACHE["nc"] = _build_nc()
    nc = _CACHE["nc"]

    wq2s = (Wq2 / np.sqrt(DK)).astype(np.float32)
    wo_bf = Wo.astype(ml_dtypes.bfloat16)
    masks = {p: _make_mask(p) for p in range(2)}
    rows = {p: _rows_sched(p) for p in range(2)}
    ones_np = np.ones((P, 1), np.float32)

    in_maps = []
    for core in range(8):
        batch, parity = core // 2, core % 2
        xb = x[batch]
        in_maps.append(
            {
                "xTkv": np.ascontiguousarray(xb.T),
                "xTq": np.ascontiguousarray(xb[rows[parity]].T),
                "wq1": Wq1,
                "wq2": wq2s,
                "wk1": Wk1,
                "wk2": Wk2,
                "wv1": Wv1,
                "wv2": Wv2,
                "wo": wo_bf,
                "maskin": masks[parity],
                "ones_in": ones_np,
            }
        )

    res = run_bass_kernel_spmd(nc, in_maps, core_ids=list(range(8)), trace=TRACE)
    LAST_RESULT = res

    out_full = np.empty((B, S, D), np.float32)
    for core in range(8):
        batch, parity = core // 2, core % 2
        out_full[batch][rows[parity]] = res.results[core]["out"]
    return out_full



# revision 2
# speedup vs baseline: 1.2146x; 1.2146x over previous
"""TRN2 Bass kernel for nn_CoreAttention_34875134444341 (v2 redesign).

Strategy (8 NeuronCores, no collectives):
  - Data-parallel over batch (4) x causal-balanced query-row split (2).
  - 8 slots of 128 query rows per core; slot k runs nb=2k+2 key blocks
    (uniform SPMD schedule; per-parity masks absorb the row-tile offset).
  - All matmul inputs bf16 (f32 PSUM accumulation): full-rate PE, half
    LDWEIGHTS time, half DMA/SBUF traffic.
  - Attention in [rows, dk] orientation with a ones-column appended to V:
    one accumulated matmul yields both attn numerator and the softmax
    denominator; normalization is a per-partition tensor_scalar on PSUM
    eviction. A 128x128 PE transpose restores [dk, rows] for the Wo stage.
  - Exp packed up to 4 score blocks per activation instruction.
  - x, qT, kT, V, attn all SBUF-resident; Wo prefetched during attention.
"""

import sys

sys.path.insert(0, "/opt/trn_rl_repo")

import numpy as np
import ml_dtypes

B, S, D = 4, 2048, 2048
H, HKV, DK = 16, 4, 128
RQ = RKV = 512
GROUP = H // HKV
P = 128
NSLOT = 8
ROWS_PER_CORE = NSLOT * P  # 1024

_CACHE = {}
TRACE = False
LAST_RESULT = None


def _rows_sched(parity):
    return np.concatenate(
        [(2 * k + parity) * P + np.arange(P) for k in range(NSLOT)]
    )


def _make_mask(parity):
    """[128 keys, 8 slots, 2 last-blocks, 2 heads, 128 rows] additive mask."""
    m = np.zeros((P, NSLOT, 2, 2, P), np.float32)
    for k in range(NSLOT):
        tile_idx = 2 * k + parity
        nb = 2 * k + 2
        rows = tile_idx * P + np.arange(P)
        for jj in range(2):
            blk = nb - 2 + jj
            keys = blk * P + np.arange(P)
            bad = keys[:, None] > rows[None, :]
            for h in range(2):
                m[:, k, jj, h, :][bad] = -1e30
    return m


def _build_nc():
    import concourse.tile as tile
    from concourse import bacc, mybir, masks as masks_mod

    f32 = mybir.dt.float32
    bf16 = mybir.dt.bfloat16
    Exp = mybir.ActivationFunctionType.Exp
    Add = mybir.AluOpType.add

    nc = bacc.Bacc("TRN2", target_bir_lowering=False, debug=False)

    xq = nc.dram_tensor("xq", [D, ROWS_PER_CORE], bf16, kind="ExternalInput")
    xkv = nc.dram_tensor("xkv", [D, S], bf16, kind="ExternalInput")
    wq1 = nc.dram_tensor("wq1", [D, RQ], bf16, kind="ExternalInput")
    wq2 = nc.dram_tensor("wq2", [RQ, H * DK], bf16, kind="ExternalInput")
    wk1 = nc.dram_tensor("wk1", [D, RKV], bf16, kind="ExternalInput")
    wk2 = nc.dram_tensor("wk2", [RKV, HKV * DK], bf16, kind="ExternalInput")
    wv1 = nc.dram_tensor("wv1", [D, RKV], bf16, kind="ExternalInput")
    wv2 = nc.dram_tensor("wv2", [RKV, HKV * DK], bf16, kind="ExternalInput")
    wo = nc.dram_tensor("wo", [D, D], bf16, kind="ExternalInput")
    maskin = nc.dram_tensor("maskin", [P, NSLOT, 2, 2, P], f32, kind="ExternalInput")
    out = nc.dram_tensor("out", [ROWS_PER_CORE, D], f32, kind="ExternalOutput")

    xq_t = xq.rearrange("(dc p) r -> p dc r", p=P)      # [128, 16, 1024]
    xkv_t = xkv.rearrange("(dc p) s -> p dc s", p=P)    # [128, 16, 2048]
    wq1_t = wq1.rearrange("(dc p) r -> p dc r", p=P)    # [128, 16, 512]
    wk1_t = wk1.rearrange("(dc p) r -> p dc r", p=P)
    wv1_t = wv1.rearrange("(dc p) r -> p dc r", p=P)
    wq2_t = wq2.rearrange("(rc p) h -> p rc h", p=P)    # [128, 4, 2048]
    wk2_t = wk2.rearrange("(rc p) h -> p rc h", p=P)    # [128, 4, 512]
    wv2_t = wv2.rearrange("(rc p) h -> p rc h", p=P)
    wo_t = wo.rearrange("(hc p) o -> p hc o", p=P)      # [128, 16, 2048]

    with tile.TileContext(nc) as tc:
        with tc.tile_pool(name="keep", bufs=1) as keep:
            ident = keep.tile([P, P], bf16)
            masks_mod.make_identity(nc, ident[:])
            qT_sb = keep.tile([P, H, ROWS_PER_CORE], bf16)
            kT_sb = keep.tile([P, HKV, S], bf16)
            v_sb = keep.tile([P, S // P, HKV, DK + 1], bf16)
            attn_all = keep.tile([P, NSLOT, H, P], bf16)
            nc.gpsimd.memset(v_sb[:, :, :, DK : DK + 1], 1.0)

            # ---------------- Phase 1: Q projection -----------------------
            with (
                tc.tile_pool(name="p1w", bufs=1) as p1w,
                tc.tile_pool(name="p1mid", bufs=1) as p1mid,
                tc.tile_pool(name="p1psA", bufs=2, space="PSUM") as p1psA,
                tc.tile_pool(name="p1psB", bufs=2, space="PSUM") as p1psB,
            ):
                wq1_sb = p1w.tile([P, 16, RQ], bf16)
                for rc in range(4):
                    nc.sync.dma_start(
                        wq1_sb[:, :, rc * P : (rc + 1) * P],
                        wq1_t[:, :, rc * P : (rc + 1) * P],
                    )
                xq_sb = p1w.tile([P, 16, ROWS_PER_CORE], bf16)
                for g in range(2):
                    nc.sync.dma_start(
                        xq_sb[:, :, g * 512 : (g + 1) * 512],
                        xq_t[:, :, g * 512 : (g + 1) * 512],
                    )
                wq2_sb = p1w.tile([P, 4, H * DK], bf16)
                nc.sync.dma_start(wq2_sb[:], wq2_t)

                q1t = p1mid.tile([P, 4, ROWS_PER_CORE], bf16)
                for g in range(2):
                    for rc in range(4):
                        ps_q = p1psA.tile([P, 512], f32, tag="q1")
                        for dc in range(16):
                            nc.tensor.matmul(
                                ps_q[:],
                                wq1_sb[:, dc, rc * P : (rc + 1) * P],
                                xq_sb[:, dc, g * 512 : (g + 1) * 512],
                                start=(dc == 0),
                                stop=(dc == 15),
                            )
                        nc.vector.tensor_copy(
                            q1t[:, rc, g * 512 : (g + 1) * 512], ps_q[:]
                        )
                for h in range(H):
                    for g in range(2):
                        ps_q2 = p1psB.tile([P, 512], f32, tag="q2")
                        for rc in range(4):
                            nc.tensor.matmul(
                                ps_q2[:],
                                wq2_sb[:, rc, h * P : (h + 1) * P],
                                q1t[:, rc, g * 512 : (g + 1) * 512],
                                start=(rc == 0),
                                stop=(rc == 3),
                            )
                        nc.vector.tensor_copy(
                            qT_sb[:, h, g * 512 : (g + 1) * 512], ps_q2[:]
                        )

            # ---------------- Phase 2: K and V projections ----------------
            with (
                tc.tile_pool(name="p2w", bufs=1) as p2w,
                tc.tile_pool(name="p2x", bufs=2) as p2x,
                tc.tile_pool(name="p2mid", bufs=2) as p2mid,
                tc.tile_pool(name="p2psA", bufs=2, space="PSUM") as p2psA,
                tc.tile_pool(name="p2psB", bufs=2, space="PSUM") as p2psB,
            ):
                w1_sb = p2w.tile([P, 2, 16, RKV], bf16)
                nc.sync.dma_start(w1_sb[:, 0], wk1_t)
                nc.sync.dma_start(w1_sb[:, 1], wv1_t)
                w2_sb = p2w.tile([P, 2, 4, HKV * DK], bf16)
                nc.sync.dma_start(w2_sb[:, 0], wk2_t)
                nc.sync.dma_start(w2_sb[:, 1], wv2_t)

                for tcn in range(4):
                    xt = p2x.tile([P, 16, 512], bf16, tag="xt")
                    nc.sync.dma_start(xt[:], xkv_t[:, :, tcn * 512 : (tcn + 1) * 512])
                    mid = p2mid.tile([P, 2, 4, 512], bf16, tag="mid")
                    for which in range(2):
                        for rc in range(4):
                            ps_m = p2psA.tile([P, 512], f32, tag="mid")
                            for dc in range(16):
                                nc.tensor.matmul(
                                    ps_m[:],
                                    w1_sb[:, which, dc, rc * P : (rc + 1) * P],
                                    xt[:, dc],
                                    start=(dc == 0),
                                    stop=(dc == 15),
                                )
                            nc.vector.tensor_copy(mid[:, which, rc], ps_m[:])
                    for hc in range(HKV):
                        ps_k = p2psB.tile([P, 512], f32, tag="w2")
                        for rc in range(4):
                            nc.tensor.matmul(
                                ps_k[:],
                                w2_sb[:, 0, rc, hc * P : (hc + 1) * P],
                                mid[:, 0, rc],
                                start=(rc == 0),
                                stop=(rc == 3),
                            )
                        nc.vector.tensor_copy(
                            kT_sb[:, hc, tcn * 512 : (tcn + 1) * 512], ps_k[:]
                        )
                    for i in range(4):
                        ps_v = p2psB.tile([P, 4, P], f32, tag="w2")
                        for rc in range(4):
                            nc.tensor.matmul(
                                ps_v[:],
                                mid[:, 1, rc, i * P : (i + 1) * P],
                                w2_sb[:, 1, rc],
                                start=(rc == 0),
                                stop=(rc == 3),
                            )
                        nc.vector.tensor_copy(
                            v_sb[:, tcn * 4 + i, :, 0:DK], ps_v[:]
                        )

            # ---------------- Phases 3+4 ----------------------------------
            with (
                tc.tile_pool(name="p3m", bufs=1) as p3m,
                tc.tile_pool(name="wo_w", bufs=1) as wo_w,
            ):
                mask_sb = p3m.tile([P, NSLOT, 2, 2, P], f32)
                nc.sync.dma_start(mask_sb[:], maskin[:])
                wo_sb = wo_w.tile([P, 16, D], bf16)
                for oc in range(4):
                    nc.sync.dma_start(
                        wo_sb[:, :, oc * 512 : (oc + 1) * 512],
                        wo_t[:, :, oc * 512 : (oc + 1) * 512],
                    )

                # -------- Phase 3: attention --------
                with (
                    tc.tile_pool(name="p3e", bufs=3) as p3e,
                    tc.tile_pool(name="p3an", bufs=6) as p3an,
                    tc.tile_pool(name="p3rec", bufs=6) as p3rec,
                    tc.tile_pool(name="sc_ps", bufs=2, space="PSUM") as sc_ps,
                    tc.tile_pool(name="at_ps", bufs=2, space="PSUM") as at_ps,
                    tc.tile_pool(name="tp_ps", bufs=2, space="PSUM") as tp_ps,
                ):
                    pending = []

                    def flush_pending():
                        while pending:
                            an_t, kk, hh = pending.pop(0)
                            tp = tp_ps.tile([P, P], bf16, tag="tp")
                            nc.tensor.transpose(tp[:], an_t[:], ident[:])
                            nc.vector.tensor_copy(attn_all[:, kk, hh, :], tp[:])

                    for k in range(NSLOT - 1, -1, -1):
                        nb = 2 * k + 2
                        npk = (nb + 3) // 4
                        for hp in range(H // 2):
                            kvh = hp // 2
                            at = at_ps.tile([P, 2, DK + 1], f32, tag="at")
                            for pk in range(npk):
                                cnt = min(4, nb - 4 * pk)
                                sc = sc_ps.tile([P, 4, 2, P], f32, tag="sc")
                                for j in range(cnt):
                                    b = 4 * pk + j
                                    nc.tensor.matmul(
                                        sc[:, j],
                                        kT_sb[:, kvh, b * P : (b + 1) * P],
                                        qT_sb[:, 2 * hp : 2 * hp + 2, k * P : (k + 1) * P],
                                        start=(j % 2 == 0),
                                        stop=(j % 2 == 1),
                                    )
                                if pk == npk - 1:
                                    for jj in range(2):
                                        j = cnt - 2 + jj
                                        nc.vector.tensor_tensor(
                                            sc[:, j], sc[:, j], mask_sb[:, k, jj], Add
                                        )
                                e = p3e.tile([P, 4, 2, P], bf16, tag="e")
                                nc.scalar.activation(e[:, :cnt], sc[:, :cnt], Exp)
                                for j in range(cnt):
                                    b = 4 * pk + j
                                    for i in range(2):
                                        nc.tensor.matmul(
                                            at[:, i, :],
                                            e[:, j, i, :],
                                            v_sb[:, b, kvh, :],
                                            start=(b == 0 and i == 0),
                                            stop=(b == nb - 1 and i == 1),
                                        )
                            newpend = []
                            for i in range(2):
                                rec = p3rec.tile([P, 1], f32, tag="rec")
                                nc.vector.reciprocal(rec[:], at[:, i, DK : DK + 1])
                                an = p3an.tile([P, P], bf16, tag="an")
                                nc.vector.tensor_scalar_mul(
                                    an[:], at[:, i, 0:DK], rec[:]
                                )
                                newpend.append((an, k, 2 * hp + i))
                            flush_pending()
                            pending.extend(newpend)
                    flush_pending()

                # -------- Phase 4: Wo --------
                with (
                    tc.tile_pool(name="p4o", bufs=3) as p4o,
                    tc.tile_pool(name="p4ps", bufs=8, space="PSUM") as p4ps,
                ):
                    for s8 in range(NSLOT):
                        pss = [
                            p4ps.tile([P, 512], f32, tag="o", name=f"ps_o{s8}_{oc}")
                            for oc in range(4)
                        ]
                        for hc in range(16):
                            for oc in range(4):
                                nc.tensor.matmul(
                                    pss[oc][:],
                                    attn_all[:, s8, hc, :],
                                    wo_sb[:, hc, oc * 512 : (oc + 1) * 512],
                                    start=(hc == 0),
                                    stop=(hc == 15),
                                )
                        for oc in range(4):
                            ob = p4o.tile([P, 512], f32, tag="ob")
                            nc.vector.tensor_copy(ob[:], pss[oc][:])
                            nc.sync.dma_start(
                                out[s8 * P : (s8 + 1) * P, oc * 512 : (oc + 1) * 512],
                                ob[:],
                            )

    nc.finalize()
    return nc


def _prep_inputs(x, Wq1, Wq2, Wk1, Wk2, Wv1, Wv2, Wo):
    bf16 = ml_dtypes.bfloat16
    wq2s = (np.asarray(Wq2, np.float32) / np.sqrt(DK)).astype(bf16)
    weights = {
        "wq1": np.asarray(Wq1, np.float32).astype(bf16),
        "wq2": wq2s,
        "wk1": np.asarray(Wk1, np.float32).astype(bf16),
        "wk2": np.asarray(Wk2, np.float32).astype(bf16),
        "wv1": np.asarray(Wv1, np.float32).astype(bf16),
        "wv2": np.asarray(Wv2, np.float32).astype(bf16),
        "wo": np.asarray(Wo, np.float32).astype(bf16),
    }
    masks = {p: _make_mask(p) for p in range(2)}
    rows = {p: _rows_sched(p) for p in range(2)}
    xT = [np.ascontiguousarray(np.asarray(x[b], np.float32).T).astype(bf16)
          for b in range(B)]
    in_maps = []
    for core in range(8):
        batch, parity = core // 2, core % 2
        m = {
            "xkv": xT[batch],
            "xq": np.ascontiguousarray(xT[batch][:, rows[parity]]),
            "maskin": masks[parity],
        }
        m.update(weights)
        in_maps.append(m)
    return in_maps, rows


def kernel(x, Wq1, Wq2, Wk1, Wk2, Wv1, Wv2, Wo):
    global LAST_RESULT
    from concourse.bass_utils import run_bass_kernel_spmd

    if "nc" not in _CACHE:
        _CACHE["nc"] = _build_nc()
    nc = _CACHE["nc"]

    in_maps, rows = _prep_inputs(x, Wq1, Wq2, Wk1, Wk2, Wv1, Wv2, Wo)
    res = run_bass_kernel_spmd(nc, in_maps, core_ids=list(range(8)), trace=TRACE)
    LAST_RESULT = res

    out_full = np.empty((B, S, D), np.float32)
    for core in range(8):
        batch, parity = core // 2, core % 2
        out_full[batch][rows[parity]] = res.results[core]["out"]
    return out_full


# revision 3
# speedup vs baseline: 1.2694x; 1.0452x over previous
"""TRN2 Bass kernel for nn_CoreAttention_34875134444341 (v2 redesign).

Strategy (8 NeuronCores, no collectives):
  - Data-parallel over batch (4) x causal-balanced query-row split (2).
  - 8 slots of 128 query rows per core; slot k runs nb=2k+2 key blocks
    (uniform SPMD schedule; per-parity masks absorb the row-tile offset).
  - All matmul inputs bf16 (f32 PSUM accumulation): full-rate PE, half
    LDWEIGHTS time, half DMA/SBUF traffic.
  - Attention in [rows, dk] orientation with a ones-column appended to V:
    one accumulated matmul yields both attn numerator and the softmax
    denominator; normalization is a per-partition tensor_scalar on PSUM
    eviction. A 128x128 PE transpose restores [dk, rows] for the Wo stage.
  - Exp packed up to 4 score blocks per activation instruction.
  - x, qT, kT, V, attn all SBUF-resident; Wo prefetched during attention.
"""

import sys

sys.path.insert(0, "/opt/trn_rl_repo")

import numpy as np
import ml_dtypes

B, S, D = 4, 2048, 2048
H, HKV, DK = 16, 4, 128
RQ = RKV = 512
GROUP = H // HKV
P = 128
NSLOT = 8
ROWS_PER_CORE = NSLOT * P  # 1024

_CACHE = {}
TRACE = False
LAST_RESULT = None


def _rows_sched(parity):
    return np.concatenate(
        [(2 * k + parity) * P + np.arange(P) for k in range(NSLOT)]
    )


def _make_mask(parity):
    """[128 keys, 8 slots, 2 last-blocks, 2 heads, 128 rows] additive mask."""
    m = np.zeros((P, NSLOT, 2, 2, P), np.float32)
    for k in range(NSLOT):
        tile_idx = 2 * k + parity
        nb = 2 * k + 2
        rows = tile_idx * P + np.arange(P)
        for jj in range(2):
            blk = nb - 2 + jj
            keys = blk * P + np.arange(P)
            bad = keys[:, None] > rows[None, :]
            for h in range(2):
                m[:, k, jj, h, :][bad] = -1e30
    return m


def _build_nc():
    import concourse.tile as tile
    from concourse import bacc, mybir, masks as masks_mod

    f32 = mybir.dt.float32
    bf16 = mybir.dt.bfloat16
    Exp = mybir.ActivationFunctionType.Exp
    Add = mybir.AluOpType.add

    nc = bacc.Bacc("TRN2", target_bir_lowering=False, debug=False)

    xq = nc.dram_tensor("xq", [D, ROWS_PER_CORE], bf16, kind="ExternalInput")
    xkv = nc.dram_tensor("xkv", [D, S], bf16, kind="ExternalInput")
    wq1 = nc.dram_tensor("wq1", [D, RQ], bf16, kind="ExternalInput")
    wq2 = nc.dram_tensor("wq2", [RQ, H * DK], bf16, kind="ExternalInput")
    wk1 = nc.dram_tensor("wk1", [D, RKV], bf16, kind="ExternalInput")
    wk2 = nc.dram_tensor("wk2", [RKV, HKV * DK], bf16, kind="ExternalInput")
    wv1 = nc.dram_tensor("wv1", [D, RKV], bf16, kind="ExternalInput")
    wv2 = nc.dram_tensor("wv2", [RKV, HKV * DK], bf16, kind="ExternalInput")
    wo = nc.dram_tensor("wo", [D, D], bf16, kind="ExternalInput")
    maskin = nc.dram_tensor("maskin", [P, NSLOT, 2, 2, P], f32, kind="ExternalInput")
    out = nc.dram_tensor("out", [ROWS_PER_CORE, D], f32, kind="ExternalOutput")

    xq_t = xq.rearrange("(dc p) r -> p dc r", p=P)      # [128, 16, 1024]
    xkv_t = xkv.rearrange("(dc p) s -> p dc s", p=P)    # [128, 16, 2048]
    wq1_t = wq1.rearrange("(dc p) r -> p dc r", p=P)    # [128, 16, 512]
    wk1_t = wk1.rearrange("(dc p) r -> p dc r", p=P)
    wv1_t = wv1.rearrange("(dc p) r -> p dc r", p=P)
    wq2_t = wq2.rearrange("(rc p) h -> p rc h", p=P)    # [128, 4, 2048]
    wk2_t = wk2.rearrange("(rc p) h -> p rc h", p=P)    # [128, 4, 512]
    wv2_t = wv2.rearrange("(rc p) h -> p rc h", p=P)
    wo_t = wo.rearrange("(hc p) o -> p hc o", p=P)      # [128, 16, 2048]

    with tile.TileContext(nc) as tc:
        with tc.tile_pool(name="keep", bufs=1) as keep:
            qT_sb = keep.tile([P, H, ROWS_PER_CORE], bf16)
            kT_sb = keep.tile([P, HKV, S], bf16)
            v_sb = keep.tile([P, S // P, HKV, DK + 1], bf16)
            attn_all = keep.tile([P, NSLOT, H, P], bf16)
            nc.gpsimd.memset(v_sb[:, :, :, DK : DK + 1], 1.0)

            # P2 weights pool opened early so their DMAs overlap P1 compute.
            with (
                tc.tile_pool(name="p2w", bufs=1) as p2w,
                tc.tile_pool(name="p1mid", bufs=1) as p1mid,
            ):
                q1t = p1mid.tile([P, 4, ROWS_PER_CORE], bf16)
                w1_sb = p2w.tile([P, 2, 16, RKV], bf16)
                w2_sb = p2w.tile([P, 2, 4, HKV * DK], bf16)

                # ------------ Phase 1a: q1 = x @ Wq1 ----------------------
                with (
                    tc.tile_pool(name="p1w", bufs=1) as p1w,
                    tc.tile_pool(name="p1psA", bufs=2, space="PSUM") as p1psA,
                ):
                    wq1_sb = p1w.tile([P, 16, RQ], bf16)
                    xq_sb = p1w.tile([P, 16, ROWS_PER_CORE], bf16)
                    # startup-critical DMA order: rc0 weights, first x half
                    nc.sync.dma_start(wq1_sb[:, :, 0:P], wq1_t[:, :, 0:P])
                    nc.sync.dma_start(xq_sb[:, :, 0:512], xq_t[:, :, 0:512])
                    for rc in range(1, 4):
                        nc.sync.dma_start(
                            wq1_sb[:, :, rc * P : (rc + 1) * P],
                            wq1_t[:, :, rc * P : (rc + 1) * P],
                        )
                    nc.sync.dma_start(xq_sb[:, :, 512:1024], xq_t[:, :, 512:1024])

                    for g in range(2):
                        for rc in range(4):
                            ps_q = p1psA.tile([P, 512], f32, tag="q1")
                            for dc in range(16):
                                nc.tensor.matmul(
                                    ps_q[:],
                                    wq1_sb[:, dc, rc * P : (rc + 1) * P],
                                    xq_sb[:, dc, g * 512 : (g + 1) * 512],
                                    start=(dc == 0),
                                    stop=(dc == 15),
                                )
                            nc.vector.tensor_copy(
                                q1t[:, rc, g * 512 : (g + 1) * 512], ps_q[:]
                            )

                # ------------ Phase 1b: qT = q1 @ Wq2 ---------------------
                with (
                    tc.tile_pool(name="p1wb", bufs=1) as p1wb,
                    tc.tile_pool(name="p1psB", bufs=2, space="PSUM") as p1psB,
                ):
                    wq2_sb = p1wb.tile([P, 4, H * DK], bf16)
                    nc.sync.dma_start(wq2_sb[:], wq2_t)
                    # prefetch P2 weights now; they land during P1 compute
                    nc.sync.dma_start(w1_sb[:, 0], wk1_t)
                    nc.sync.dma_start(w1_sb[:, 1], wv1_t)
                    nc.sync.dma_start(w2_sb[:, 0], wk2_t)
                    nc.sync.dma_start(w2_sb[:, 1], wv2_t)

                    for h in range(H):
                        for g in range(2):
                            ps_q2 = p1psB.tile([P, 512], f32, tag="q2")
                            for rc in range(4):
                                nc.tensor.matmul(
                                    ps_q2[:],
                                    wq2_sb[:, rc, h * P : (h + 1) * P],
                                    q1t[:, rc, g * 512 : (g + 1) * 512],
                                    start=(rc == 0),
                                    stop=(rc == 3),
                                )
                            nc.vector.tensor_copy(
                                qT_sb[:, h, g * 512 : (g + 1) * 512], ps_q2[:]
                            )

                # ------------ Phase 2: K and V projections ----------------
                with (
                    tc.tile_pool(name="p2x", bufs=2) as p2x,
                    tc.tile_pool(name="p2mid", bufs=2) as p2mid,
                    tc.tile_pool(name="p2psA", bufs=2, space="PSUM") as p2psA,
                    tc.tile_pool(name="p2psB", bufs=2, space="PSUM") as p2psB,
                ):
                    for tcn in range(4):
                        xt = p2x.tile([P, 16, 512], bf16, tag="xt")
                        nc.sync.dma_start(
                            xt[:], xkv_t[:, :, tcn * 512 : (tcn + 1) * 512]
                        )
                        mid = p2mid.tile([P, 2, 4, 512], bf16, tag="mid")
                        for which in range(2):
                            for rc in range(4):
                                ps_m = p2psA.tile([P, 512], f32, tag="mid")
                                for dc in range(16):
                                    nc.tensor.matmul(
                                        ps_m[:],
                                        w1_sb[:, which, dc, rc * P : (rc + 1) * P],
                                        xt[:, dc],
                                        start=(dc == 0),
                                        stop=(dc == 15),
                                    )
                                nc.vector.tensor_copy(mid[:, which, rc], ps_m[:])
                        for hc in range(HKV):
                            ps_k = p2psB.tile([P, 512], f32, tag="w2")
                            for rc in range(4):
                                nc.tensor.matmul(
                                    ps_k[:],
                                    w2_sb[:, 0, rc, hc * P : (hc + 1) * P],
                                    mid[:, 0, rc],
                                    start=(rc == 0),
                                    stop=(rc == 3),
                                )
                            nc.vector.tensor_copy(
                                kT_sb[:, hc, tcn * 512 : (tcn + 1) * 512], ps_k[:]
                            )
                        for i in range(4):
                            ps_v = p2psB.tile([P, 4, P], f32, tag="w2")
                            for rc in range(4):
                                nc.tensor.matmul(
                                    ps_v[:],
                                    mid[:, 1, rc, i * P : (i + 1) * P],
                                    w2_sb[:, 1, rc],
                                    start=(rc == 0),
                                    stop=(rc == 3),
                                )
                            nc.vector.tensor_copy(
                                v_sb[:, tcn * 4 + i, :, 0:DK], ps_v[:]
                            )

            # ---------------- Phases 3+4 ----------------------------------
            with (
                tc.tile_pool(name="p3m", bufs=1) as p3m,
                tc.tile_pool(name="wo_w", bufs=1) as wo_w,
            ):
                ident = p3m.tile([P, P], bf16)
                masks_mod.make_identity(nc, ident[:])
                mask_sb = p3m.tile([P, NSLOT, 2, 2, P], f32)
                nc.sync.dma_start(mask_sb[:], maskin[:])
                wo_sb = wo_w.tile([P, 16, D], bf16)
                for oc in range(4):
                    nc.sync.dma_start(
                        wo_sb[:, :, oc * 512 : (oc + 1) * 512],
                        wo_t[:, :, oc * 512 : (oc + 1) * 512],
                    )

                # -------- Phase 3: attention (flat software pipeline) -----
                with (
                    tc.tile_pool(name="p3e", bufs=3) as p3e,
                    tc.tile_pool(name="p3an", bufs=6) as p3an,
                    tc.tile_pool(name="p3rec", bufs=6) as p3rec,
                    tc.tile_pool(name="sc_ps", bufs=2, space="PSUM") as sc_ps,
                    tc.tile_pool(name="at_ps", bufs=2, space="PSUM") as at_ps,
                    tc.tile_pool(name="tp_ps", bufs=2, space="PSUM") as tp_ps,
                ):
                    # flat pack list: (k, hp, pk, cnt, is_first, is_last)
                    packs = []
                    for k in range(NSLOT - 1, -1, -1):
                        nb = 2 * k + 2
                        npk = (nb + 3) // 4
                        for hp in range(H // 2):
                            for pk in range(npk):
                                cnt = min(4, nb - 4 * pk)
                                packs.append(
                                    (k, hp, pk, cnt, pk == 0, pk == npk - 1)
                                )

                    at_tiles = {}      # hp-key -> at psum tile
                    pending_attv = None  # (e, k, hp, pk, cnt, first, last)
                    pending_tp = []    # deferred transposes (an, k, h)

                    def emit_attv(item):
                        e_t, k, hp, pk, cnt, first, last = item
                        nb = 2 * k + 2
                        kvh = hp // 2
                        if first:
                            at_tiles[(k, hp)] = at_ps.tile(
                                [P, 2, DK + 1], f32, tag="at", name=f"at_{k}_{hp}"
                            )
                        at = at_tiles[(k, hp)]
                        for j in range(cnt):
                            b = 4 * pk + j
                            for i in range(2):
                                nc.tensor.matmul(
                                    at[:, i, :],
                                    e_t[:, j, i, :],
                                    v_sb[:, b, kvh, :],
                                    start=(b == 0 and i == 0),
                                    stop=(b == nb - 1 and i == 1),
                                )
                        if last:
                            # normalize both heads; defer transposes one hp
                            while pending_tp:
                                an_t, kk, hh = pending_tp.pop(0)
                                tp = tp_ps.tile([P, P], bf16, tag="tp")
                                nc.tensor.transpose(tp[:], an_t[:], ident[:])
                                nc.vector.tensor_copy(
                                    attn_all[:, kk, hh, :], tp[:]
                                )
                            at = at_tiles.pop((k, hp))
                            for i in range(2):
                                rec = p3rec.tile([P, 1], f32, tag="rec")
                                nc.vector.reciprocal(rec[:], at[:, i, DK : DK + 1])
                                an = p3an.tile([P, P], bf16, tag="an")
                                nc.vector.tensor_scalar_mul(
                                    an[:], at[:, i, 0:DK], rec[:]
                                )
                                pending_tp.append((an, k, 2 * hp + i))

                    for k, hp, pk, cnt, first, last in packs:
                        kvh = hp // 2
                        sc = sc_ps.tile([P, 4, 2, P], f32, tag="sc")
                        for j in range(cnt):
                            b = 4 * pk + j
                            nc.tensor.matmul(
                                sc[:, j],
                                kT_sb[:, kvh, b * P : (b + 1) * P],
                                qT_sb[:, 2 * hp : 2 * hp + 2, k * P : (k + 1) * P],
                                start=(j % 2 == 0),
                                stop=(j % 2 == 1),
                            )
                        if last:
                            nc.vector.tensor_tensor(
                                sc[:, cnt - 2 : cnt],
                                sc[:, cnt - 2 : cnt],
                                mask_sb[:, k],
                                Add,
                            )
                        e = p3e.tile([P, 4, 2, P], bf16, tag="e")
                        nc.scalar.activation(e[:, :cnt], sc[:, :cnt], Exp)
                        if pending_attv is not None:
                            emit_attv(pending_attv)
                        pending_attv = (e, k, hp, pk, cnt, first, last)
                    emit_attv(pending_attv)
                    while pending_tp:
                        an_t, kk, hh = pending_tp.pop(0)
                        tp = tp_ps.tile([P, P], bf16, tag="tp")
                        nc.tensor.transpose(tp[:], an_t[:], ident[:])
                        nc.vector.tensor_copy(attn_all[:, kk, hh, :], tp[:])

                # -------- Phase 4: Wo --------
                with (
                    tc.tile_pool(name="p4o", bufs=3) as p4o,
                    tc.tile_pool(name="p4ps", bufs=8, space="PSUM") as p4ps,
                ):
                    for s8 in range(NSLOT):
                        pss = [
                            p4ps.tile([P, 512], f32, tag="o", name=f"ps_o{s8}_{oc}")
                            for oc in range(4)
                        ]
                        for hc in range(16):
                            for oc in range(4):
                                nc.tensor.matmul(
                                    pss[oc][:],
                                    attn_all[:, s8, hc, :],
                                    wo_sb[:, hc, oc * 512 : (oc + 1) * 512],
                                    start=(hc == 0),
                                    stop=(hc == 15),
                                )
                        for oc in range(4):
                            ob = p4o.tile([P, 512], f32, tag="ob")
                            nc.vector.tensor_copy(ob[:], pss[oc][:])
                            nc.sync.dma_start(
                                out[s8 * P : (s8 + 1) * P, oc * 512 : (oc + 1) * 512],
                                ob[:],
                            )

    nc.finalize()
    return nc


def _prep_inputs(x, Wq1, Wq2, Wk1, Wk2, Wv1, Wv2, Wo):
    bf16 = ml_dtypes.bfloat16
    wq2s = (np.asarray(Wq2, np.float32) / np.sqrt(DK)).astype(bf16)
    weights = {
        "wq1": np.asarray(Wq1, np.float32).astype(bf16),
        "wq2": wq2s,
        "wk1": np.asarray(Wk1, np.float32).astype(bf16),
        "wk2": np.asarray(Wk2, np.float32).astype(bf16),
        "wv1": np.asarray(Wv1, np.float32).astype(bf16),
        "wv2": np.asarray(Wv2, np.float32).astype(bf16),
        "wo": np.asarray(Wo, np.float32).astype(bf16),
    }
    masks = {p: _make_mask(p) for p in range(2)}
    rows = {p: _rows_sched(p) for p in range(2)}
    xT = [np.ascontiguousarray(np.asarray(x[b], np.float32).T).astype(bf16)
          for b in range(B)]
    in_maps = []
    for core in range(8):
        batch, parity = core // 2, core % 2
        m = {
            "xkv": xT[batch],
            "xq": np.ascontiguousarray(xT[batch][:, rows[parity]]),
            "maskin": masks[parity],
        }
        m.update(weights)
        in_maps.append(m)
    return in_maps, rows


def kernel(x, Wq1, Wq2, Wk1, Wk2, Wv1, Wv2, Wo):
    global LAST_RESULT
    from concourse.bass_utils import run_bass_kernel_spmd

    if "nc" not in _CACHE:
        _CACHE["nc"] = _build_nc()
    nc = _CACHE["nc"]

    in_maps, rows = _prep_inputs(x, Wq1, Wq2, Wk1, Wk2, Wv1, Wv2, Wo)
    res = run_bass_kernel_spmd(nc, in_maps, core_ids=list(range(8)), trace=TRACE)
    LAST_RESULT = res

    out_full = np.empty((B, S, D), np.float32)
    for core in range(8):
        batch, parity = core // 2, core % 2
        out_full[batch][rows[parity]] = res.results[core]["out"]
    return out_full


# revision 4
# speedup vs baseline: 1.3771x; 1.0848x over previous
"""TRN2 Bass kernel for nn_CoreAttention_34875134444341 (v2 redesign).

Strategy (8 NeuronCores, no collectives):
  - Data-parallel over batch (4) x causal-balanced query-row split (2).
  - 8 slots of 128 query rows per core; slot k runs nb=2k+2 key blocks
    (uniform SPMD schedule; per-parity masks absorb the row-tile offset).
  - All matmul inputs bf16 (f32 PSUM accumulation): full-rate PE, half
    LDWEIGHTS time, half DMA/SBUF traffic.
  - Attention in [rows, dk] orientation with a ones-column appended to V:
    one accumulated matmul yields both attn numerator and the softmax
    denominator; normalization is a per-partition tensor_scalar on PSUM
    eviction. A 128x128 PE transpose restores [dk, rows] for the Wo stage.
  - Exp packed up to 4 score blocks per activation instruction.
  - x, qT, kT, V, attn all SBUF-resident; Wo prefetched during attention.
"""

import sys

sys.path.insert(0, "/opt/trn_rl_repo")

import numpy as np
import ml_dtypes

B, S, D = 4, 2048, 2048
H, HKV, DK = 16, 4, 128
RQ = RKV = 512
GROUP = H // HKV
P = 128
NSLOT = 8
ROWS_PER_CORE = NSLOT * P  # 1024

_CACHE = {}
TRACE = False
LAST_RESULT = None


def _rows_sched(parity):
    return np.concatenate(
        [(2 * k + parity) * P + np.arange(P) for k in range(NSLOT)]
    )


def _make_mask(parity):
    """[128 keys, 8 slots, 2 last-blocks, 2 heads, 128 rows] additive mask."""
    m = np.zeros((P, NSLOT, 2, 2, P), np.float32)
    for k in range(NSLOT):
        tile_idx = 2 * k + parity
        nb = 2 * k + 2
        rows = tile_idx * P + np.arange(P)
        for jj in range(2):
            blk = nb - 2 + jj
            keys = blk * P + np.arange(P)
            bad = keys[:, None] > rows[None, :]
            for h in range(2):
                m[:, k, jj, h, :][bad] = -1e30
    return m.astype(ml_dtypes.bfloat16)


def _build_nc():
    import concourse.tile as tile
    from concourse import bacc, mybir, masks as masks_mod

    f32 = mybir.dt.float32
    bf16 = mybir.dt.bfloat16
    Exp = mybir.ActivationFunctionType.Exp
    Add = mybir.AluOpType.add

    nc = bacc.Bacc("TRN2", target_bir_lowering=False, debug=False)

    xq = nc.dram_tensor("xq", [D, ROWS_PER_CORE], bf16, kind="ExternalInput")
    xkv = nc.dram_tensor("xkv", [D, S], bf16, kind="ExternalInput")
    wq1 = nc.dram_tensor("wq1", [D, RQ], bf16, kind="ExternalInput")
    wq2 = nc.dram_tensor("wq2", [RQ, H * DK], bf16, kind="ExternalInput")
    wk1 = nc.dram_tensor("wk1", [D, RKV], bf16, kind="ExternalInput")
    wk2 = nc.dram_tensor("wk2", [RKV, HKV * DK], bf16, kind="ExternalInput")
    wv1 = nc.dram_tensor("wv1", [D, RKV], bf16, kind="ExternalInput")
    wv2 = nc.dram_tensor("wv2", [RKV, HKV * DK], bf16, kind="ExternalInput")
    wo = nc.dram_tensor("wo", [D, D], bf16, kind="ExternalInput")
    maskin = nc.dram_tensor("maskin", [P, NSLOT, 2, 2, P], bf16, kind="ExternalInput")
    out = nc.dram_tensor("out", [ROWS_PER_CORE, D], f32, kind="ExternalOutput")

    xq_t = xq.rearrange("(dc p) r -> p dc r", p=P)      # [128, 16, 1024]
    xkv_t = xkv.rearrange("(dc p) s -> p dc s", p=P)    # [128, 16, 2048]
    wq1_t = wq1.rearrange("(dc p) r -> p dc r", p=P)    # [128, 16, 512]
    wk1_t = wk1.rearrange("(dc p) r -> p dc r", p=P)
    wv1_t = wv1.rearrange("(dc p) r -> p dc r", p=P)
    wq2_t = wq2.rearrange("(rc p) h -> p rc h", p=P)    # [128, 4, 2048]
    wk2_t = wk2.rearrange("(rc p) h -> p rc h", p=P)    # [128, 4, 512]
    wv2_t = wv2.rearrange("(rc p) h -> p rc h", p=P)
    wo_t = wo.rearrange("(hc p) o -> p hc o", p=P)      # [128, 16, 2048]

    with tile.TileContext(nc) as tc:
        with tc.tile_pool(name="keep", bufs=1) as keep:
            qT_sb = keep.tile([P, H, ROWS_PER_CORE], bf16)
            kT_sb = keep.tile([P, HKV, S], bf16)
            v_sb = keep.tile([P, S // P, HKV, DK + 1], bf16)
            attn_all = keep.tile([P, NSLOT, H, P], bf16)
            nc.gpsimd.memset(v_sb[:, :, :, DK : DK + 1], 1.0)

            # P2 weights pool opened early so their DMAs overlap P1 compute.
            with (
                tc.tile_pool(name="p2w", bufs=1) as p2w,
                tc.tile_pool(name="p1mid", bufs=1) as p1mid,
            ):
                q1t = p1mid.tile([P, 4, ROWS_PER_CORE], bf16)
                wq2_sb = p2w.tile([P, 4, H * DK], bf16)
                w1_sb = p2w.tile([P, 2, 16, RKV], bf16)
                w2_sb = p2w.tile([P, 2, 4, HKV * DK], bf16)

                # ------------ Phase 1a: q1 = x @ Wq1 ----------------------
                with (
                    tc.tile_pool(name="p1w", bufs=1) as p1w,
                    tc.tile_pool(name="p1x", bufs=2) as p1x,
                    tc.tile_pool(name="p1psA", bufs=2, space="PSUM") as p1psA,
                ):
                    wq1_sb = p1w.tile([P, 16, RQ], bf16)
                    # startup-critical DMA order: rc0 weights, first x chunks
                    nc.sync.dma_start(wq1_sb[:, :, 0:P], wq1_t[:, :, 0:P])
                    xq_tiles = []
                    for g4 in range(2):
                        xq_c = p1x.tile([P, 16, 256], bf16, tag="xq", name=f"xq{g4}")
                        xq_tiles.append(xq_c)
                        nc.sync.dma_start(
                            xq_c[:], xq_t[:, :, g4 * 256 : (g4 + 1) * 256]
                        )
                        if g4 == 0:
                            for rc in range(1, 4):
                                nc.sync.dma_start(
                                    wq1_sb[:, :, rc * P : (rc + 1) * P],
                                    wq1_t[:, :, rc * P : (rc + 1) * P],
                                )
                    # prefetch P1b + P2 weights; they land during P1 compute.
                    # (emitted before xq chunks 2/3, whose slot-reuse waits
                    # would otherwise block the in-order DMA queue)
                    nc.sync.dma_start(wq2_sb[:], wq2_t)
                    nc.sync.dma_start(w1_sb[:, 0], wk1_t)
                    nc.sync.dma_start(w1_sb[:, 1], wv1_t)
                    nc.sync.dma_start(w2_sb[:, 0], wk2_t)
                    nc.sync.dma_start(w2_sb[:, 1], wv2_t)
                    for g4 in range(2, 4):
                        xq_c = p1x.tile([P, 16, 256], bf16, tag="xq", name=f"xq{g4}")
                        xq_tiles.append(xq_c)
                        nc.sync.dma_start(
                            xq_c[:], xq_t[:, :, g4 * 256 : (g4 + 1) * 256]
                        )

                    for g4 in range(4):
                        for rc in range(4):
                            ps_q = p1psA.tile([P, 256], f32, tag="q1")
                            for dc in range(16):
                                nc.tensor.matmul(
                                    ps_q[:],
                                    wq1_sb[:, dc, rc * P : (rc + 1) * P],
                                    xq_tiles[g4][:, dc],
                                    start=(dc == 0),
                                    stop=(dc == 15),
                                )
                            nc.vector.tensor_copy(
                                q1t[:, rc, g4 * 256 : (g4 + 1) * 256], ps_q[:]
                            )

                # ------------ Phase 1b: qT = q1 @ Wq2 ---------------------
                with (
                    tc.tile_pool(name="p1psB", bufs=2, space="PSUM") as p1psB,
                ):
                    for h in range(H):
                        for g in range(2):
                            ps_q2 = p1psB.tile([P, 512], f32, tag="q2")
                            for rc in range(4):
                                nc.tensor.matmul(
                                    ps_q2[:],
                                    wq2_sb[:, rc, h * P : (h + 1) * P],
                                    q1t[:, rc, g * 512 : (g + 1) * 512],
                                    start=(rc == 0),
                                    stop=(rc == 3),
                                )
                            nc.vector.tensor_copy(
                                qT_sb[:, h, g * 512 : (g + 1) * 512], ps_q2[:]
                            )

                # ------------ Phase 2: K and V projections ----------------
                with (
                    tc.tile_pool(name="p2x", bufs=2) as p2x,
                    tc.tile_pool(name="p2mid", bufs=1) as p2mid,
                    tc.tile_pool(name="p2psA", bufs=2, space="PSUM") as p2psA,
                    tc.tile_pool(name="p2psB", bufs=2, space="PSUM") as p2psB,
                ):
                    for tcn in range(4):
                        xt = p2x.tile([P, 16, 512], bf16, tag="xt")
                        nc.sync.dma_start(
                            xt[:], xkv_t[:, :, tcn * 512 : (tcn + 1) * 512]
                        )
                        mid = p2mid.tile([P, 2, 4, 512], bf16, tag="mid")
                        for which in range(2):
                            for rc in range(4):
                                ps_m = p2psA.tile([P, 512], f32, tag="mid")
                                for dc in range(16):
                                    nc.tensor.matmul(
                                        ps_m[:],
                                        w1_sb[:, which, dc, rc * P : (rc + 1) * P],
                                        xt[:, dc],
                                        start=(dc == 0),
                                        stop=(dc == 15),
                                    )
                                nc.vector.tensor_copy(mid[:, which, rc], ps_m[:])
                        for hc in range(HKV):
                            ps_k = p2psB.tile([P, 512], f32, tag="w2")
                            for rc in range(4):
                                nc.tensor.matmul(
                                    ps_k[:],
                                    w2_sb[:, 0, rc, hc * P : (hc + 1) * P],
                                    mid[:, 0, rc],
                                    start=(rc == 0),
                                    stop=(rc == 3),
                                )
                            nc.vector.tensor_copy(
                                kT_sb[:, hc, tcn * 512 : (tcn + 1) * 512], ps_k[:]
                            )
                        for i in range(4):
                            ps_v = p2psB.tile([P, 4, P], f32, tag="w2")
                            for rc in range(4):
                                nc.tensor.matmul(
                                    ps_v[:],
                                    mid[:, 1, rc, i * P : (i + 1) * P],
                                    w2_sb[:, 1, rc],
                                    start=(rc == 0),
                                    stop=(rc == 3),
                                )
                            nc.vector.tensor_copy(
                                v_sb[:, tcn * 4 + i, :, 0:DK], ps_v[:]
                            )

            # ---------------- Phases 3+4 ----------------------------------
            with (
                tc.tile_pool(name="p3m", bufs=1) as p3m,
                tc.tile_pool(name="wo_w", bufs=1) as wo_w,
            ):
                ident = p3m.tile([P, P], bf16)
                masks_mod.make_identity(nc, ident[:])
                mask_sb = p3m.tile([P, NSLOT, 2, 2, P], bf16)
                nc.sync.dma_start(mask_sb[:], maskin[:])
                wo_sb = wo_w.tile([P, 16, D], bf16)
                for oc in range(4):
                    nc.sync.dma_start(
                        wo_sb[:, :, oc * 512 : (oc + 1) * 512],
                        wo_t[:, :, oc * 512 : (oc + 1) * 512],
                    )

                # -------- Phase 3 + interleaved Phase 4 -------------------
                with (
                    tc.tile_pool(name="p3e", bufs=3) as p3e,
                    tc.tile_pool(name="p3an", bufs=6) as p3an,
                    tc.tile_pool(name="p3rec", bufs=6) as p3rec,
                    tc.tile_pool(name="p4o", bufs=3) as p4o,
                    tc.tile_pool(name="sc_ps", bufs=2, space="PSUM") as sc_ps,
                    tc.tile_pool(name="at_ps", bufs=2, space="PSUM") as at_ps,
                    tc.tile_pool(name="tp_ps", bufs=1, space="PSUM") as tp_ps,
                    tc.tile_pool(name="wo_ps", bufs=1, space="PSUM") as wo_ps,
                ):
                    # flat pack list: (k, hp, pk, cnt, is_first, is_last)
                    packs = []
                    for k in range(NSLOT - 1, -1, -1):
                        nb = 2 * k + 2
                        npk = (nb + 3) // 4
                        for hp in range(H // 2):
                            for pk in range(npk):
                                cnt = min(4, nb - 4 * pk)
                                packs.append(
                                    (k, hp, pk, cnt, pk == 0, pk == npk - 1)
                                )

                    at_tiles = {}        # hp-key -> at psum tile
                    pending_attv = None  # (e, k, hp, pk, cnt, first, last)
                    pending_tp = []      # deferred transposes (an, k, h)
                    slot_copies_left = {k: H for k in range(NSLOT)}

                    # -- interleaved Wo machinery --
                    wo_queue = []        # slot generators awaiting pumping
                    wo_cur = [None]

                    def wo_slot_gen(k):
                        for oc in range(4):
                            ps_o = wo_ps.tile(
                                [P, 512], f32, tag="wo", name=f"wops_{k}_{oc}"
                            )
                            for hc in range(16):
                                nc.tensor.matmul(
                                    ps_o[:],
                                    attn_all[:, k, hc, :],
                                    wo_sb[:, hc, oc * 512 : (oc + 1) * 512],
                                    start=(hc == 0),
                                    stop=(hc == 15),
                                )
                                yield
                            ob = p4o.tile([P, 512], f32, tag="ob", name=f"ob{k}_{oc}")
                            nc.vector.tensor_copy(ob[:], ps_o[:])
                            nc.sync.dma_start(
                                out[k * P : (k + 1) * P, oc * 512 : (oc + 1) * 512],
                                ob[:],
                            )
                            yield

                    def wo_pump(n):
                        emitted = 0
                        while emitted < n:
                            if wo_cur[0] is None:
                                if not wo_queue:
                                    return
                                wo_cur[0] = wo_queue.pop(0)
                            try:
                                next(wo_cur[0])
                                emitted += 1
                            except StopIteration:
                                wo_cur[0] = None

                    def emit_tp_copy(an_t, kk, hh):
                        tp = tp_ps.tile([P, P], bf16, tag="tp")
                        nc.tensor.transpose(tp[:], an_t[:], ident[:])
                        nc.vector.tensor_copy(attn_all[:, kk, hh, :], tp[:])
                        slot_copies_left[kk] -= 1
                        if slot_copies_left[kk] == 0:
                            wo_queue.append(wo_slot_gen(kk))

                    def emit_attv(item):
                        e_t, k, hp, pk, cnt, first, last = item
                        nb = 2 * k + 2
                        kvh = hp // 2
                        if first:
                            at_tiles[(k, hp)] = at_ps.tile(
                                [P, 2, DK + 1], f32, tag="at", name=f"at_{k}_{hp}"
                            )
                        at = at_tiles[(k, hp)]
                        for j in range(cnt):
                            b = 4 * pk + j
                            for i in range(2):
                                nc.tensor.matmul(
                                    at[:, i, :],
                                    e_t[:, j, i, :],
                                    v_sb[:, b, kvh, :],
                                    start=(b == 0 and i == 0),
                                    stop=(b == nb - 1 and i == 1),
                                )
                        if last:
                            # normalize both heads; defer transposes one hp
                            while pending_tp:
                                emit_tp_copy(*pending_tp.pop(0))
                            at = at_tiles.pop((k, hp))
                            for i in range(2):
                                rec = p3rec.tile([P, 1], f32, tag="rec")
                                nc.vector.reciprocal(rec[:], at[:, i, DK : DK + 1])
                                an = p3an.tile([P, P], bf16, tag="an")
                                nc.vector.tensor_scalar_mul(
                                    an[:], at[:, i, 0:DK], rec[:]
                                )
                                pending_tp.append((an, k, 2 * hp + i))

                    for k, hp, pk, cnt, first, last in packs:
                        kvh = hp // 2
                        sc = sc_ps.tile([P, 4, 2, P], f32, tag="sc")
                        for j in range(cnt):
                            b = 4 * pk + j
                            stop_flag = (j % 2 == 1) and not (last and j >= cnt - 2)
                            nc.tensor.matmul(
                                sc[:, j],
                                kT_sb[:, kvh, b * P : (b + 1) * P],
                                qT_sb[:, 2 * hp : 2 * hp + 2, k * P : (k + 1) * P],
                                start=(j % 2 == 0),
                                stop=stop_flag,
                            )
                        if last:
                            # causal mask folded in as a PE accumulation:
                            # sc[last 2 blocks] += identity.T @ mask
                            nc.tensor.matmul(
                                sc[:, cnt - 2 : cnt],
                                ident[:],
                                mask_sb[:, k],
                                start=False,
                                stop=True,
                            )
                        e = p3e.tile([P, 4, 2, P], bf16, tag="e")
                        nc.scalar.activation(e[:, :cnt], sc[:, :cnt], Exp)
                        if pending_attv is not None:
                            emit_attv(pending_attv)
                        pending_attv = (e, k, hp, pk, cnt, first, last)
                        wo_pump(1)
                    emit_attv(pending_attv)
                    while pending_tp:
                        emit_tp_copy(*pending_tp.pop(0))
                    wo_pump(1 << 30)

    nc.finalize()
    return nc


def _prep_inputs(x, Wq1, Wq2, Wk1, Wk2, Wv1, Wv2, Wo):
    bf16 = ml_dtypes.bfloat16
    wq2s = (np.asarray(Wq2, np.float32) / np.sqrt(DK)).astype(bf16)
    weights = {
        "wq1": np.asarray(Wq1, np.float32).astype(bf16),
        "wq2": wq2s,
        "wk1": np.asarray(Wk1, np.float32).astype(bf16),
        "wk2": np.asarray(Wk2, np.float32).astype(bf16),
        "wv1": np.asarray(Wv1, np.float32).astype(bf16),
        "wv2": np.asarray(Wv2, np.float32).astype(bf16),
        "wo": np.asarray(Wo, np.float32).astype(bf16),
    }
    masks = {p: _make_mask(p) for p in range(2)}
    rows = {p: _rows_sched(p) for p in range(2)}
    xT = [np.ascontiguousarray(np.asarray(x[b], np.float32).T).astype(bf16)
          for b in range(B)]
    in_maps = []
    for core in range(8):
        batch, parity = core // 2, core % 2
        m = {
            "xkv": xT[batch],
            "xq": np.ascontiguousarray(xT[batch][:, rows[parity]]),
            "maskin": masks[parity],
        }
        m.update(weights)
        in_maps.append(m)
    return in_maps, rows


def kernel(x, Wq1, Wq2, Wk1, Wk2, Wv1, Wv2, Wo):
    global LAST_RESULT
    from concourse.bass_utils import run_bass_kernel_spmd

    if "nc" not in _CACHE:
        _CACHE["nc"] = _build_nc()
    nc = _CACHE["nc"]

    in_maps, rows = _prep_inputs(x, Wq1, Wq2, Wk1, Wk2, Wv1, Wv2, Wo)
    res = run_bass_kernel_spmd(nc, in_maps, core_ids=list(range(8)), trace=TRACE)
    LAST_RESULT = res

    out_full = np.empty((B, S, D), np.float32)
    for core in range(8):
        batch, parity = core // 2, core % 2
        out_full[batch][rows[parity]] = res.results[core]["out"]
    return out_full
